# revision 10
# baseline (speedup 1.0000x reference)
"""Bidirectional 2-layer RNN (B=64, T=1024, NIN=H=512) on 8 Trainium2 cores.

Sharding: 4 core-pairs x {fwd, bwd}. Pair p owns sequences [16p, 16p+16);
the even core of the pair runs the forward direction, the odd core the
backward direction (on host-time-reversed inputs, so the device program is
identical SPMD). Layer-0 outputs are exchanged pairwise with chunked
AllGathers that pipeline behind the layer-0 scan.

On-device layout is [hidden, time, batch] so the recurrent matmuls keep the
weights stationary ([128,128] bf16 tiles) and the tanh output feeds the next
step's moving operand with no transposes.

v2 changes vs v1:
- pre-activations are bf16 end to end; the identity used to inject them into
  PSUM is bf16 too (fp32 stationaries disable FWL and pay ~2x LDWEIGHTS).
- scan matmuls are ordered so the 8 MMs gated by the previous step's FIRST
  act (k0,k1) run before the 8 gated by its LAST act (k2,k3); the two tanh
  acts issue as early as possible. This hides most of the ~640ns PSUM->ACT
  ->SBUF round-trip behind PE work.
- the projection GEMM no longer round-trips through DRAM: proj output stage
  tiles are consumed directly by the scan from SBUF.
- proj matmuls are emitted one at a time between scan steps (instead of
  16-MM bursts between chunks), filling the PE idle window inside each step
  so the projection rides the scan's dependency slack for free.
"""

import sys

sys.path.insert(0, "/opt/trn_rl_repo")

import numpy as np
import ml_dtypes

from contextlib import ExitStack

import concourse.bacc as bacc
import concourse.mybir as mybir
from concourse.tile import TileContext
from concourse.bass_utils import run_bass_kernel_spmd

BF16 = mybir.dt.bfloat16
F32 = mybir.dt.float32
ACT_TANH = mybir.ActivationFunctionType.Tanh

B, T_FULL, NIN, H = 64, 1024, 512, 512
NCORES, NPAIRS, BL = 8, 4, 16  # cores, pairs, sequences per pair
P = 128
KT, MT = H // P, H // P  # 4 k-tiles, 4 m-tiles
RG = [[0, 1], [2, 3], [4, 5], [6, 7]]  # pair replica groups

CH = 64  # scan steps per chunk (= out0 exchange chunk)
PT = 16  # proj t-steps per chunk (moving N = PT*BL = 256)
RP = CH // PT  # proj chunks per scan chunk
LEAD = RP  # proj chunks emitted before the scan starts
DMA_AHEAD = 5  # mv chunks prefetched ahead of their matmuls


def _load_weight(nc, pool, name):
    """Host layout [512, 512] (k-major) -> SBUF [128, KT, MT, 128] bf16."""
    dram = nc.declare_dram_parameter(name, [H, H], BF16, isOutput=False)
    w = pool.tile([P, KT, MT, P], BF16, tag=name)
    for k in range(KT):
        nc.sync.dma_start(out=w[:, k, :, :], in_=dram[P * k : P * (k + 1), :])
    return w


def _load_bias(nc, pool, name):
    dram = nc.declare_dram_parameter(name, [P, MT], F32, isOutput=False)
    b = pool.tile([P, MT], F32, tag=name)
    nc.sync.dma_start(out=b[:], in_=dram[:])
    return b


class ProjEmitter:
    """Input-projection GEMM for one layer, emitted one matmul at a time.

    Chunk pc covers time steps [pc*PT, (pc+1)*PT). Finished stage tiles
    (bf16 pre-activations, [P, MT, PT, BL]) land in self.stages for the
    scan to consume straight from SBUF.
    """

    def __init__(self, nc, pstp, ppsp, n_chunks, load_chunk, weights, bias):
        self.nc = nc
        self.pstp, self.ppsp = pstp, ppsp
        self.n_chunks = n_chunks
        self.load_chunk = load_chunk  # pc -> list of mv tiles (emits DMAs)
        self.weights = weights  # k loop: for w in weights: for k in range(KT)
        self.bias = bias
        self.NK = KT * len(weights)
        self.stages = {}  # pc -> ready stage tile
        self.mvs = {}  # pc -> mv tiles
        self.pc = 0
        self.m = 0
        self.k = 0
        self.ps = None
        self.stage = None
        self.credit = 0.0
        if n_chunks > 0:
            self.mvs[0] = load_chunk(0)

    def done(self):
        return self.pc >= self.n_chunks

    def emit_mm(self):
        """Emit one projection matmul (plus any boundary work)."""
        nc = self.nc
        if self.done():
            return False
        if self.m == 0 and self.k == 0:
            # Deep DMA prefetch: the Tile scheduler places instructions on a
            # simulated timeline; a late simulated DMA makes a whole chunk's
            # matmuls "ready" at once and they get scheduled as one burst
            # that stalls the scan chain. Issuing loads several chunks early
            # lets the scheduler spread the proj matmuls into per-step gaps.
            for ahead in range(1, DMA_AHEAD + 1):
                pc2 = self.pc + ahead
                if pc2 < self.n_chunks and pc2 not in self.mvs:
                    self.mvs[pc2] = self.load_chunk(pc2)
            if self.stage is None:
                self.stage = self.pstp.tile([P, MT, PT, BL], BF16, tag="pst")
        if self.ps is None:
            self.ps = self.ppsp.tile([P, PT * BL], F32, tag="proj_ps")
        src, k = divmod(self.k, KT)
        w = self.weights[src]
        mv = self.mvs[self.pc][src]
        nc.tensor.matmul(
            self.ps[:], w[:, k, self.m, :], mv[:, k, :, :],
            start=(self.k == 0), stop=(self.k == self.NK - 1),
        )
        self.k += 1
        if self.k == self.NK:
            nc.vector.tensor_scalar_add(
                self.stage[:, self.m, :, :],
                self.ps[:],
                self.bias[:, self.m : self.m + 1],
            )
            self.ps = None
            self.k = 0
            self.m += 1
            if self.m == MT:
                self.stages[self.pc] = self.stage
                del self.mvs[self.pc]
                self.m = 0
                self.stage = None
                self.pc += 1
        return True

    def emit_chunks(self, n_target):
        while self.pc < min(n_target, self.n_chunks):
            self.emit_mm()

    def fill(self, rate):
        """Emit proj MMs at `rate` per scan step (fractional, accumulated)."""
        self.credit += rate
        while self.credit >= 1.0:
            self.credit -= 1.0
            if not self.emit_mm():
                self.credit = 0.0
                break


def _run_scan(nc, st, whh, ident, out_writer, proj, rate, T):
    """The full T-step recurrent tanh scan for one layer, with proj fill."""
    NCH = T // CH
    stag_pool, ps_pool = st["pools"]
    stages = proj.stages
    prev_stag = None
    for ch in range(NCH):
        t0 = ch * CH
        stag = stag_pool.tile([P, KT, CH, BL], BF16, tag="sstag")
        for t in range(CH):
            s = t0 + t
            pc, tt = divmod(s, PT)
            pre = stages[pc]
            hp = None
            if s > 0:
                hp = stag[:, :, t - 1, :] if t > 0 else prev_stag[:, :, CH - 1, :]
            psA = ps_pool.tile([P, 2 * BL], F32, tag="psA")
            psB = ps_pool.tile([P, 2 * BL], F32, tag="psB")
            # inject pre-activations (bf16 identity keeps FWL enabled)
            nc.tensor.matmul(
                psA[:], ident[:], pre[:, 0:2, tt, :], start=True, stop=(s == 0)
            )
            nc.tensor.matmul(
                psB[:], ident[:], pre[:, 2:4, tt, :], start=True, stop=(s == 0)
            )
            if s == 0:
                nc.scalar.activation(stag[:, 0:2, t, :], psA[:], ACT_TANH)
                nc.scalar.activation(stag[:, 2:4, t, :], psB[:], ACT_TANH)
                proj.fill(rate)
            else:
                # k0/k1 matmuls of both banks are gated by actA(t-1), which
                # is already in flight; they run while actB(t-1) drains.
                for ps_h, ms in ((psA, (0, 1)), (psB, (2, 3))):
                    for k in (0, 1):
                        for mi, m in enumerate(ms):
                            sl = slice(mi * BL, (mi + 1) * BL)
                            nc.tensor.matmul(
                                ps_h[:, sl], whh[:, k, m, :], hp[:, k, :],
                                start=False, stop=False,
                            )
                # proj matmuls sit in the PE's mid-step idle window (waiting
                # for actB(t-1)), ahead of the act semaphore thresholds.
                proj.fill(rate)
                # k2/k3 (gated by actB(t-1)); each bank's act issues right
                # after its own last matmul so its semaphore prefix stops
                # there.
                for ps_h, ms in ((psA, (0, 1)), (psB, (2, 3))):
                    for k in (2, 3):
                        for mi, m in enumerate(ms):
                            sl = slice(mi * BL, (mi + 1) * BL)
                            nc.tensor.matmul(
                                ps_h[:, sl], whh[:, k, m, :], hp[:, k, :],
                                start=False,
                                stop=(k == 3 and mi == 1),
                            )
                    nc.scalar.activation(
                        stag[:, ms[0] : ms[0] + 2, t, :], ps_h[:], ACT_TANH
                    )
        out_writer(ch, t0, stag)
        # stages for this chunk are consumed; allow pool reuse
        for pc in range(RP * ch, RP * (ch + 1)):
            stages.pop(pc, None)
        prev_stag = stag


def build_nc(T, dbg=False):
    NCH = T // CH
    NPC = T // PT
    nc = bacc.Bacc(num_devices=NCORES)

    xT = nc.declare_dram_parameter("xT", [NIN, T, BL], BF16, isOutput=False)
    identp = nc.declare_dram_parameter("ident", [P, P], BF16, isOutput=False)
    out1T = nc.declare_dram_parameter("out1T", [H, T, BL], BF16, isOutput=True)
    sel = nc.declare_dram_parameter("sel", [1, 2], mybir.dt.uint32, isOutput=False)

    out0C = nc.dram_tensor("out0C", [NCH, H, CH, BL], BF16)
    both0 = nc.dram_tensor("both0", [NCH, 2, H, CH, BL], BF16)

    with TileContext(nc) as tc:
        with tc.tile_pool(name="const", bufs=1) as cpool:
            wih0 = _load_weight(nc, cpool, "wih0T")
            whh0 = _load_weight(nc, cpool, "whh0T")
            wih1o = _load_weight(nc, cpool, "wih1ownT")
            wih1x = _load_weight(nc, cpool, "wothT")
            whh1 = _load_weight(nc, cpool, "whh1T")
            bias0 = _load_bias(nc, cpool, "bias0")
            bias1 = _load_bias(nc, cpool, "bias1")
            ident = cpool.tile([P, P], BF16, tag="ident")
            nc.sync.dma_start(out=ident[:], in_=identp[:])
            sel_sb = cpool.tile([1, 2], mybir.dt.uint32, tag="sel")
            nc.sync.dma_start(out=sel_sb[:], in_=sel[:])
            va = nc.values_load(
                sel_sb[0:1, 0:1], min_val=0, max_val=1,
                skip_runtime_bounds_check=True,
            )
            vb = nc.values_load(
                sel_sb[0:1, 1:2], min_val=0, max_val=1,
                skip_runtime_bounds_check=True,
            )

            stack = ExitStack()
            mvp = stack.enter_context(tc.tile_pool(name="mv", bufs=7))
            ppsp = stack.enter_context(tc.tile_pool(name="pps", bufs=2, space="PSUM"))
            pstp = stack.enter_context(tc.tile_pool(name="pst", bufs=10))
            sstagp = stack.enter_context(tc.tile_pool(name="sstag", bufs=3))
            spsp = stack.enter_context(tc.tile_pool(name="sps", bufs=2, space="PSUM"))

            def load0(pc):
                t0 = pc * PT
                mv = mvp.tile([P, KT, PT, BL], BF16, tag="mv0")
                for kb in range(KT):
                    nc.sync.dma_start(
                        out=mv[:, kb, :, :],
                        in_=xT[P * kb : P * (kb + 1), t0 : t0 + PT, :],
                    )
                return [mv]

            def load1(pc):
                t0 = pc * PT
                mvA = mvp.tile([P, KT, PT, BL], BF16, tag="mv1a")
                c0, o0 = divmod(t0, CH)
                for kb in range(KT):
                    nc.sync.dma_start(
                        out=mvA[:, kb, :, :],
                        in_=out0C[c0, P * kb : P * (kb + 1), o0 : o0 + PT, :],
                    )
                mvB = mvp.tile([P, KT, PT, BL], BF16, tag="mv1b")
                pl = T - t0 - PT  # partner-time start of the flipped slab
                c1, o1 = divmod(pl, CH)
                for kb in range(KT):
                    for sslot, cond in ((0, vb), (1, va)):
                        nc.sync.dma_start(
                            out=mvB[:, kb, ::-1, :],
                            in_=both0[c1, sslot, P * kb : P * (kb + 1), o1 : o1 + PT, :],
                            cond=cond,
                        )
                return [mvA, mvB]

            def w_out0(ch, t0, stag):
                for k in range(KT):
                    nc.sync.dma_start(
                        out=out0C[ch, P * k : P * (k + 1), :, :],
                        in_=stag[:, k, :, :],
                    )
                nc.gpsimd.collective_compute(
                    "AllGather",
                    mybir.AluOpType.bypass,
                    replica_groups=RG,
                    ins=[out0C[ch].rearrange("h t b -> (h t b)")],
                    outs=[both0[ch].rearrange("s h t b -> (s h t b)")],
                )

            def w_out1(ch, t0, stag):
                for k in range(KT):
                    nc.sync.dma_start(
                        out=out1T[P * k : P * (k + 1), t0 : t0 + CH, :],
                        in_=stag[:, k, :, :],
                    )

            # ---- layer 0 ----
            proj0 = ProjEmitter(nc, pstp, ppsp, NPC, load0, [wih0], bias0)
            proj0.emit_chunks(LEAD)
            st0 = {"pools": (sstagp, spsp)}
            rate0 = RP * MT * proj0.NK / CH  # proj MMs per scan step
            _run_scan(nc, st0, whh0, ident, w_out0, proj0, rate0, T)

            if dbg:
                out0dbg = nc.declare_dram_parameter(
                    "out0dbg", [NCH, H, CH, BL], BF16, isOutput=True
                )
                nc.sync.dma_start(out=out0dbg[:], in_=out0C[:])

            # ---- layer 1 ----
            proj1 = ProjEmitter(
                nc, pstp, ppsp, NPC, load1, [wih1o, wih1x], bias1
            )
            proj1.emit_chunks(LEAD)
            st1 = {"pools": (sstagp, spsp)}
            rate1 = RP * MT * proj1.NK / CH
            _run_scan(nc, st1, whh1, ident, w_out1, proj1, rate1, T)

            stack.close()

    if not nc.is_finalized():
        nc.finalize()
    return nc


def _bf16(a):
    return np.ascontiguousarray(a).astype(ml_dtypes.bfloat16)


def make_in_maps(inputs, T):
    x = np.asarray(inputs["input_feat"])  # [B, T, NIN] f32
    maps = []
    for p in range(NPAIRS):
        seqs = slice(BL * p, BL * (p + 1))
        for par, d in ((0, "f"), (1, "b")):
            xs = x[seqs, :T]
            if par == 1:
                xs = xs[:, ::-1]
            col = slice(0, H) if par == 0 else slice(H, 2 * H)
            ocol = slice(H, 2 * H) if par == 0 else slice(0, H)
            w1 = np.asarray(inputs[f"w_ih_1{d}"])
            m = {
                "xT": _bf16(xs.transpose(2, 1, 0)),
                "ident": _bf16(np.eye(P, dtype=np.float32)),
                "wih0T": _bf16(np.asarray(inputs[f"w_ih_0{d}"]).T),
                "whh0T": _bf16(np.asarray(inputs[f"w_hh_0{d}"]).T),
                "wih1ownT": _bf16(w1[:, col].T),
                "wothT": _bf16(w1[:, ocol].T),
                "whh1T": _bf16(np.asarray(inputs[f"w_hh_1{d}"]).T),
                "bias0": np.ascontiguousarray(
                    (np.asarray(inputs[f"b_ih_0{d}"]) + np.asarray(inputs[f"b_hh_0{d}"]))
                    .reshape(MT, P).T.astype(np.float32)
                ),
                "bias1": np.ascontiguousarray(
                    (np.asarray(inputs[f"b_ih_1{d}"]) + np.asarray(inputs[f"b_hh_1{d}"]))
                    .reshape(MT, P).T.astype(np.float32)
                ),
                "sel": np.array([[1 - par, par]], dtype=np.uint32),
            }
            maps.append(m)
    return maps


def assemble_output(results, T):
    y = np.empty((B, T, 2 * H), dtype=np.float32)
    for p in range(NPAIRS):
        seqs = slice(BL * p, BL * (p + 1))
        for par in (0, 1):
            o = np.asarray(results[2 * p + par]["out1T"]).astype(np.float32)
            o = o.transpose(2, 1, 0)  # [BL, T, H]
            if par == 1:
                o = o[:, ::-1]
            y[seqs, :, par * H : (par + 1) * H] = o
    return y


def run(inputs, T=T_FULL, trace=False, trace_cores=None):
    nc = build_nc(T)
    in_maps = make_in_maps(inputs, T)
    res = run_bass_kernel_spmd(
        nc, in_maps, list(range(NCORES)), trace=trace, trace_cores=trace_cores
    )
    return assemble_output(res.results, T), res


def kernel(**inputs):
    out, _ = run(inputs, T=T_FULL, trace=False)
    return out


# revision 16
# speedup vs baseline: 1.1172x; 1.1172x over previous
"""Bidirectional 2-layer RNN (B=64, T=1024, NIN=H=512) on 8 Trainium2 cores.

Sharding: 4 core-pairs x {fwd, bwd}. Pair p owns sequences [16p, 16p+16);
the even core of the pair runs the forward direction, the odd core the
backward direction (on host-time-reversed inputs, so the device program is
identical SPMD). Layer-0 outputs are exchanged pairwise with chunked
AllGathers that pipeline behind the layer-0 scan.

On-device layout is [hidden, time, batch] so the recurrent matmuls keep the
weights stationary ([128,128] bf16 tiles) and the tanh output feeds the next
step's moving operand with no transposes.

v2 changes vs v1:
- pre-activations are bf16 end to end; the identity used to inject them into
  PSUM is bf16 too (fp32 stationaries disable FWL and pay ~2x LDWEIGHTS).
- scan matmuls are ordered so the 8 MMs gated by the previous step's FIRST
  act (k0,k1) run before the 8 gated by its LAST act (k2,k3); the two tanh
  acts issue as early as possible. This hides most of the ~640ns PSUM->ACT
  ->SBUF round-trip behind PE work.
- the projection GEMM no longer round-trips through DRAM: proj output stage
  tiles are consumed directly by the scan from SBUF.
- proj matmuls are emitted one at a time between scan steps (instead of
  16-MM bursts between chunks), filling the PE idle window inside each step
  so the projection rides the scan's dependency slack for free.
"""

import sys

sys.path.insert(0, "/opt/trn_rl_repo")

import numpy as np
import ml_dtypes

from contextlib import ExitStack

import concourse.bacc as bacc
import concourse.mybir as mybir
from concourse.tile import TileContext
from concourse.bass_utils import run_bass_kernel_spmd

BF16 = mybir.dt.bfloat16
F32 = mybir.dt.float32
ACT_TANH = mybir.ActivationFunctionType.Tanh

B, T_FULL, NIN, H = 64, 1024, 512, 512
NCORES, NPAIRS, BL = 8, 4, 16  # cores, pairs, sequences per pair
P = 128
KT, MT = H // P, H // P  # 4 k-tiles, 4 m-tiles
RG = [[0, 1], [2, 3], [4, 5], [6, 7]]  # pair replica groups

CH = 64  # scan steps per chunk (= out0 exchange chunk)
PT = 32  # proj t-steps per chunk (moving N = PT*BL = 512)
RP = CH // PT  # proj chunks per scan chunk
LEAD = RP  # proj chunks emitted before the scan starts
DMA_AHEAD = 3  # mv chunks prefetched ahead of their matmuls
PAD_TARGET = 2  # PE-busy filler matmuls per scan step (HAM stays warm)


def _load_weight(nc, pool, name):
    """Host layout [512, 512] (k-major) -> SBUF [128, KT, MT, 128] bf16."""
    dram = nc.declare_dram_parameter(name, [H, H], BF16, isOutput=False)
    w = pool.tile([P, KT, MT, P], BF16, tag=name)
    for k in range(KT):
        nc.sync.dma_start(out=w[:, k, :, :], in_=dram[P * k : P * (k + 1), :])
    return w


def _load_bias(nc, pool, name):
    dram = nc.declare_dram_parameter(name, [P, MT], F32, isOutput=False)
    b = pool.tile([P, MT], F32, tag=name)
    nc.sync.dma_start(out=b[:], in_=dram[:])
    return b


class ProjEmitter:
    """Input-projection GEMM for one layer, emitted one matmul at a time.

    Chunk pc covers time steps [pc*PT, (pc+1)*PT). Finished stage tiles
    (bf16 pre-activations, [P, MT, PT, BL]) land in self.stages for the
    scan to consume straight from SBUF.
    """

    def __init__(self, nc, pstp, ppsp, n_chunks, load_chunk, weights, bias):
        self.nc = nc
        self.pstp, self.ppsp = pstp, ppsp
        self.n_chunks = n_chunks
        self.load_chunk = load_chunk  # pc -> list of mv tiles (emits DMAs)
        self.weights = weights  # k loop: for w in weights: for k in range(KT)
        self.bias = bias
        self.NK = KT * len(weights)
        self.stages = {}  # pc -> ready stage tile
        self.mvs = {}  # pc -> mv tiles
        self.pc = 0
        self.m = 0
        self.k = 0
        self.ps = None
        self.stage = None
        self.credit = 0.0
        if n_chunks > 0:
            self.mvs[0] = load_chunk(0)

    def done(self):
        return self.pc >= self.n_chunks

    def emit_mm(self):
        """Emit one projection matmul (plus any boundary work)."""
        nc = self.nc
        if self.done():
            return False
        if self.m == 0 and self.k == 0:
            # Deep DMA prefetch: the Tile scheduler places instructions on a
            # simulated timeline; a late simulated DMA makes a whole chunk's
            # matmuls "ready" at once and they get scheduled as one burst
            # that stalls the scan chain. Issuing loads several chunks early
            # lets the scheduler spread the proj matmuls into per-step gaps.
            for ahead in range(1, DMA_AHEAD + 1):
                pc2 = self.pc + ahead
                if pc2 < self.n_chunks and pc2 not in self.mvs:
                    self.mvs[pc2] = self.load_chunk(pc2)
            if self.stage is None:
                self.stage = self.pstp.tile([P, MT, PT, BL], BF16, tag="pst")
        if self.ps is None:
            self.ps = self.ppsp.tile([P, PT * BL], F32, tag="proj_ps")
        src, k = divmod(self.k, KT)
        w = self.weights[src]
        mv = self.mvs[self.pc][src]
        nc.tensor.matmul(
            self.ps[:], w[:, k, self.m, :], mv[:, k, :, :],
            start=(self.k == 0), stop=(self.k == self.NK - 1),
        )
        self.k += 1
        if self.k == self.NK:
            nc.vector.tensor_scalar_add(
                self.stage[:, self.m, :, :],
                self.ps[:],
                self.bias[:, self.m : self.m + 1],
            )
            self.ps = None
            self.k = 0
            self.m += 1
            if self.m == MT:
                self.stages[self.pc] = self.stage
                del self.mvs[self.pc]
                self.m = 0
                self.stage = None
                self.pc += 1
        return True

    def emit_chunks(self, n_target):
        while self.pc < min(n_target, self.n_chunks):
            self.emit_mm()

    def fill(self, rate):
        """Emit proj MMs at `rate` per scan step (fractional, accumulated).

        Returns the number of matmuls emitted this call."""
        n = 0
        self.credit += rate
        while self.credit >= 1.0:
            self.credit -= 1.0
            if not self.emit_mm():
                self.credit = 0.0
                break
            n += 1
        return n


def _run_scan(nc, st, whh, ident, out_writer, proj, rate, T):
    """The full T-step recurrent tanh scan for one layer, with proj fill."""
    NCH = T // CH
    stag_pool, ps_pool, pad_pool = st["pools"]

    def pad(n):
        # Filler matmuls (tiny 2-col stationary, discarded output) that sit
        # in the PE's per-step wait window. They keep the PE's HAM activity
        # monitor above its busy threshold so the clock holds 2.4GHz instead
        # of throttling to 1.2GHz between matmul groups.
        for _ in range(n):
            sc = pad_pool.tile([2, P], F32, tag="pad")
            nc.tensor.matmul(sc[:], ident[:, 0:2], ident[:], start=True, stop=True)
    stages = proj.stages
    prev_stag = None
    for ch in range(NCH):
        t0 = ch * CH
        stag = stag_pool.tile([P, KT, CH, BL], BF16, tag="sstag")
        for t in range(CH):
            s = t0 + t
            pc, tt = divmod(s, PT)
            pre = stages[pc]
            hp = None
            if s > 0:
                hp = stag[:, :, t - 1, :] if t > 0 else prev_stag[:, :, CH - 1, :]
            psA = ps_pool.tile([P, 2 * BL], F32, tag="psA")
            psB = ps_pool.tile([P, 2 * BL], F32, tag="psB")
            # inject pre-activations (bf16 identity keeps FWL enabled)
            nc.tensor.matmul(
                psA[:], ident[:], pre[:, 0:2, tt, :], start=True, stop=(s == 0)
            )
            nc.tensor.matmul(
                psB[:], ident[:], pre[:, 2:4, tt, :], start=True, stop=(s == 0)
            )
            if s == 0:
                nc.scalar.activation(stag[:, 0:2, t, :], psA[:], ACT_TANH)
                nc.scalar.activation(stag[:, 2:4, t, :], psB[:], ACT_TANH)
                proj.fill(rate)
            else:
                # k0/k1 matmuls of both banks are gated by actA(t-1), which
                # is already in flight; they run while actB(t-1) drains.
                for ps_h, ms in ((psA, (0, 1)), (psB, (2, 3))):
                    for k in (0, 1):
                        for mi, m in enumerate(ms):
                            sl = slice(mi * BL, (mi + 1) * BL)
                            nc.tensor.matmul(
                                ps_h[:, sl], whh[:, k, m, :], hp[:, k, :],
                                start=False, stop=False,
                            )
                # proj matmuls sit in the PE's mid-step idle window (waiting
                # for actB(t-1)), ahead of the act semaphore thresholds.
                emitted = proj.fill(rate)
                pad(max(0, PAD_TARGET - emitted))
                # k2/k3 (gated by actB(t-1)); each bank's act issues right
                # after its own last matmul so its semaphore prefix stops
                # there.
                for ps_h, ms in ((psA, (0, 1)), (psB, (2, 3))):
                    for k in (2, 3):
                        for mi, m in enumerate(ms):
                            sl = slice(mi * BL, (mi + 1) * BL)
                            nc.tensor.matmul(
                                ps_h[:, sl], whh[:, k, m, :], hp[:, k, :],
                                start=False,
                                stop=(k == 3 and mi == 1),
                            )
                    nc.scalar.activation(
                        stag[:, ms[0] : ms[0] + 2, t, :], ps_h[:], ACT_TANH
                    )
        out_writer(ch, t0, stag)
        # stages for this chunk are consumed; allow pool reuse
        for pc in range(RP * ch, RP * (ch + 1)):
            stages.pop(pc, None)
        prev_stag = stag


def build_nc(T, dbg=False):
    NCH = T // CH
    NPC = T // PT
    nc = bacc.Bacc(num_devices=NCORES)

    xT = nc.declare_dram_parameter("xT", [NIN, T, BL], BF16, isOutput=False)
    identp = nc.declare_dram_parameter("ident", [P, P], BF16, isOutput=False)
    out1T = nc.declare_dram_parameter("out1T", [H, T, BL], BF16, isOutput=True)
    sel = nc.declare_dram_parameter("sel", [1, 2], mybir.dt.uint32, isOutput=False)

    out0C = nc.dram_tensor("out0C", [NCH, H, CH, BL], BF16)
    both0 = nc.dram_tensor("both0", [NCH, 2, H, CH, BL], BF16)

    with TileContext(nc) as tc:
        with tc.tile_pool(name="const", bufs=1) as cpool:
            wih0 = _load_weight(nc, cpool, "wih0T")
            whh0 = _load_weight(nc, cpool, "whh0T")
            wih1o = _load_weight(nc, cpool, "wih1ownT")
            wih1x = _load_weight(nc, cpool, "wothT")
            whh1 = _load_weight(nc, cpool, "whh1T")
            bias0 = _load_bias(nc, cpool, "bias0")
            bias1 = _load_bias(nc, cpool, "bias1")
            ident = cpool.tile([P, P], BF16, tag="ident")
            nc.sync.dma_start(out=ident[:], in_=identp[:])
            sel_sb = cpool.tile([1, 2], mybir.dt.uint32, tag="sel")
            nc.sync.dma_start(out=sel_sb[:], in_=sel[:])
            va = nc.values_load(
                sel_sb[0:1, 0:1], min_val=0, max_val=1,
                skip_runtime_bounds_check=True,
            )
            vb = nc.values_load(
                sel_sb[0:1, 1:2], min_val=0, max_val=1,
                skip_runtime_bounds_check=True,
            )

            stack = ExitStack()
            mvp = stack.enter_context(tc.tile_pool(name="mv", bufs=5))
            ppsp = stack.enter_context(tc.tile_pool(name="pps", bufs=2, space="PSUM"))
            pstp = stack.enter_context(tc.tile_pool(name="pst", bufs=7))
            sstagp = stack.enter_context(tc.tile_pool(name="sstag", bufs=3))
            spsp = stack.enter_context(tc.tile_pool(name="sps", bufs=2, space="PSUM"))
            padp = stack.enter_context(tc.tile_pool(name="padp", bufs=2, space="PSUM"))

            def load0(pc):
                t0 = pc * PT
                mv = mvp.tile([P, KT, PT, BL], BF16, tag="mv0")
                for kb in range(KT):
                    nc.sync.dma_start(
                        out=mv[:, kb, :, :],
                        in_=xT[P * kb : P * (kb + 1), t0 : t0 + PT, :],
                    )
                return [mv]

            def load1(pc):
                t0 = pc * PT
                mvA = mvp.tile([P, KT, PT, BL], BF16, tag="mv1a")
                c0, o0 = divmod(t0, CH)
                for kb in range(KT):
                    nc.sync.dma_start(
                        out=mvA[:, kb, :, :],
                        in_=out0C[c0, P * kb : P * (kb + 1), o0 : o0 + PT, :],
                    )
                mvB = mvp.tile([P, KT, PT, BL], BF16, tag="mv1b")
                pl = T - t0 - PT  # partner-time start of the flipped slab
                c1, o1 = divmod(pl, CH)
                for kb in range(KT):
                    for sslot, cond in ((0, vb), (1, va)):
                        nc.sync.dma_start(
                            out=mvB[:, kb, ::-1, :],
                            in_=both0[c1, sslot, P * kb : P * (kb + 1), o1 : o1 + PT, :],
                            cond=cond,
                        )
                return [mvA, mvB]

            def w_out0(ch, t0, stag):
                for k in range(KT):
                    nc.sync.dma_start(
                        out=out0C[ch, P * k : P * (k + 1), :, :],
                        in_=stag[:, k, :, :],
                    )
                nc.gpsimd.collective_compute(
                    "AllGather",
                    mybir.AluOpType.bypass,
                    replica_groups=RG,
                    ins=[out0C[ch].rearrange("h t b -> (h t b)")],
                    outs=[both0[ch].rearrange("s h t b -> (s h t b)")],
                )

            def w_out1(ch, t0, stag):
                for k in range(KT):
                    nc.sync.dma_start(
                        out=out1T[P * k : P * (k + 1), t0 : t0 + CH, :],
                        in_=stag[:, k, :, :],
                    )

            # ---- layer 0 ----
            proj0 = ProjEmitter(nc, pstp, ppsp, NPC, load0, [wih0], bias0)
            proj0.emit_chunks(LEAD)
            st0 = {"pools": (sstagp, spsp, padp)}
            rate0 = RP * MT * proj0.NK / CH  # proj MMs per scan step
            _run_scan(nc, st0, whh0, ident, w_out0, proj0, rate0, T)

            if dbg:
                out0dbg = nc.declare_dram_parameter(
                    "out0dbg", [NCH, H, CH, BL], BF16, isOutput=True
                )
                nc.sync.dma_start(out=out0dbg[:], in_=out0C[:])

            # ---- layer 1 ----
            proj1 = ProjEmitter(
                nc, pstp, ppsp, NPC, load1, [wih1o, wih1x], bias1
            )
            proj1.emit_chunks(LEAD)
            st1 = {"pools": (sstagp, spsp, padp)}
            rate1 = RP * MT * proj1.NK / CH
            _run_scan(nc, st1, whh1, ident, w_out1, proj1, rate1, T)

            stack.close()

    if not nc.is_finalized():
        nc.finalize()
    return nc


def _bf16(a):
    return np.ascontiguousarray(a).astype(ml_dtypes.bfloat16)


def make_in_maps(inputs, T):
    x = np.asarray(inputs["input_feat"])  # [B, T, NIN] f32
    maps = []
    for p in range(NPAIRS):
        seqs = slice(BL * p, BL * (p + 1))
        for par, d in ((0, "f"), (1, "b")):
            xs = x[seqs, :T]
            if par == 1:
                xs = xs[:, ::-1]
            col = slice(0, H) if par == 0 else slice(H, 2 * H)
            ocol = slice(H, 2 * H) if par == 0 else slice(0, H)
            w1 = np.asarray(inputs[f"w_ih_1{d}"])
            m = {
                "xT": _bf16(xs.transpose(2, 1, 0)),
                "ident": _bf16(np.eye(P, dtype=np.float32)),
                "wih0T": _bf16(np.asarray(inputs[f"w_ih_0{d}"]).T),
                "whh0T": _bf16(np.asarray(inputs[f"w_hh_0{d}"]).T),
                "wih1ownT": _bf16(w1[:, col].T),
                "wothT": _bf16(w1[:, ocol].T),
                "whh1T": _bf16(np.asarray(inputs[f"w_hh_1{d}"]).T),
                "bias0": np.ascontiguousarray(
                    (np.asarray(inputs[f"b_ih_0{d}"]) + np.asarray(inputs[f"b_hh_0{d}"]))
                    .reshape(MT, P).T.astype(np.float32)
                ),
                "bias1": np.ascontiguousarray(
                    (np.asarray(inputs[f"b_ih_1{d}"]) + np.asarray(inputs[f"b_hh_1{d}"]))
                    .reshape(MT, P).T.astype(np.float32)
                ),
                "sel": np.array([[1 - par, par]], dtype=np.uint32),
            }
            maps.append(m)
    return maps


def assemble_output(results, T):
    y = np.empty((B, T, 2 * H), dtype=np.float32)
    for p in range(NPAIRS):
        seqs = slice(BL * p, BL * (p + 1))
        for par in (0, 1):
            o = np.asarray(results[2 * p + par]["out1T"]).astype(np.float32)
            o = o.transpose(2, 1, 0)  # [BL, T, H]
            if par == 1:
                o = o[:, ::-1]
            y[seqs, :, par * H : (par + 1) * H] = o
    return y


def run(inputs, T=T_FULL, trace=False, trace_cores=None):
    nc = build_nc(T)
    in_maps = make_in_maps(inputs, T)
    res = run_bass_kernel_spmd(
        nc, in_maps, list(range(NCORES)), trace=trace, trace_cores=trace_cores
    )
    return assemble_output(res.results, T), res


def kernel(**inputs):
    out, _ = run(inputs, T=T_FULL, trace=False)
    return out
